# revision 6
# baseline (speedup 1.0000x reference)
"""Trainium2 Bass kernel for causal multi-head attention (v2).

Problem: B=2, S=2048, D=1024, H=16 heads (hd=64), fp32 in/out.
  qkv = x @ Wqkv + bqkv ; per-head causal softmax attention ; out = ctx @ Wo + bo

Sharding (8 NeuronCores): tensor-parallel over heads - 2 heads per core.
Each core computes q/k/v projections for its 2 heads (both batches), causal
attention, and its ctx^T slice [128 feat, B*S]. Four AllToAlls (one per
(batch, half), emitted as soon as the half's ctx is done) hand each core the
[1024 feat, 128 token] block it needs for its output-projection rows; each
core computes 4 chunks of 128 output rows with the full Wo.

v2 changes vs v1:
- AllToAll instead of AllGather: 256KB per collective instead of 3.5MB.
- Output projection emitted eagerly per (batch, half) instead of at the end.
- Scores/exp/mask trimmed to the causal support at 128-column granularity
  (diagonal k-tiles only stream q-cols that can attend them).
- PSUM regrouped: 2x[128,1024] score/exp groups + 4x[128,512] small tiles,
  scores packed so each exp ACT op covers one contiguous written region.

Numerics: bf16 matmul operands, fp32 PSUM accumulation. Softmax uses
exp without max-subtraction (scores ~N(0,1) after the folded 1/sqrt(hd)
scale). The softmax denominator comes from a ones-column appended to v.
"""

import numpy as np
import ml_dtypes

B, S, D, H, NC = 2, 2048, 1024, 16, 8
HD = D // H            # 64
HPC = H // NC          # 2 heads per core
BS = B * S             # 4096
KC = D // 128          # 8 contraction chunks
SC = BS // 512         # 8 s-chunks for qkv projection
NQT = S // 512         # 4 q-windows (512) per batch
NKT = S // 128         # 16 k-tiles (128) per batch
EXPW = 2048 * 3 + 1280  # exp tile max width (window j=3)

BF16 = ml_dtypes.bfloat16

_CACHE = {}


def _build_program():
    import concourse.bass as bass
    import concourse.mybir as mybir
    from concourse import bacc
    from concourse.tile import TileContext

    dt = mybir.dt
    f32, bf16 = dt.float32, dt.bfloat16
    ALU = mybir.AluOpType
    ACTF = mybir.ActivationFunctionType

    nc = bacc.Bacc("TRN2", target_bir_lowering=False, debug=False, num_devices=NC)

    xT = nc.dram_tensor("xT", [D, BS], bf16, kind="ExternalInput")
    wqk = nc.dram_tensor("wqk", [D, 256], bf16, kind="ExternalInput")
    wv = nc.dram_tensor("wv", [D, 128], bf16, kind="ExternalInput")
    wo = nc.dram_tensor("wo", [D, D], bf16, kind="ExternalInput")
    bqk = nc.dram_tensor("bqk", [128, 2], f32, kind="ExternalInput")
    bv = nc.dram_tensor("bv", [128, 128], bf16, kind="ExternalInput")
    bo = nc.dram_tensor("bo", [128, D], f32, kind="ExternalInput")
    mask = nc.dram_tensor("mask", [128, 128], bf16, kind="ExternalInput")
    out = nc.dram_tensor("out", [4 * 128, D], f32, kind="ExternalOutput")

    # per-(batch,half) AllToAll buffers: block d of a2a_in goes to core d;
    # block d of a2a_out arrived from core d. [NC*128 feat-rows, 128 tokens].
    NCH = 2 * B
    a2a_in = [nc.dram_tensor(f"a2ain{g}", [NC * 128, 128], bf16)
              for g in range(NCH)]
    a2a_out = [nc.dram_tensor(f"a2aout{g}", [NC * 128, 128], bf16)
               for g in range(NCH)]

    with TileContext(nc) as tc:
        with (
            tc.tile_pool(name="const", bufs=1) as cpool,
            tc.tile_pool(name="big", bufs=1) as bigpool,
            tc.tile_pool(name="xstream", bufs=2) as xpool,
            tc.tile_pool(name="exp", bufs=2) as epool,
            tc.tile_pool(name="small", bufs=3) as spool,
            tc.tile_pool(name="a2a", bufs=4) as apool,
            tc.tile_pool(name="outp", bufs=4) as opool,
            tc.tile_pool(name="psA", bufs=2, space="PSUM") as psA,   # 2x [128,1024]
            tc.tile_pool(name="psB", bufs=3, space="PSUM") as psB,   # 3x [128,512]
            tc.tile_pool(name="psP", bufs=1, space="PSUM") as psP,   # 1x [128,512]
        ):
            # ---- constants / weights to SBUF ----
            # Weights ride the Scalar queue so the x-stream (Sync queue) is
            # never stuck behind them; wo (2MB, needed late) goes last.
            wqk_sb = cpool.tile([128, KC, 256], bf16, tag="wqk")
            nc.scalar.dma_start(wqk_sb[:], wqk.rearrange("(ko p) m -> p ko m", p=128))
            wv_sb = cpool.tile([128, KC, 128], bf16, tag="wv")
            nc.scalar.dma_start(wv_sb[:], wv.rearrange("(ko p) m -> p ko m", p=128))
            bqk_sb = cpool.tile([128, 2], f32, tag="bqk")
            nc.scalar.dma_start(bqk_sb[:], bqk[:])
            bv_sb = cpool.tile([128, 128], bf16, tag="bv")
            nc.scalar.dma_start(bv_sb[:], bv[:])
            bo_sb = cpool.tile([128, D], f32, tag="bo")
            nc.scalar.dma_start(bo_sb[:], bo[:])
            mask_sb = cpool.tile([128, 128], bf16, tag="mask")
            nc.scalar.dma_start(mask_sb[:], mask[:])
            wo_sb = cpool.tile([128, KC, D], bf16, tag="wo")
            nc.gpsimd.dma_start(wo_sb[:], wo.rearrange("(ko p) m -> p ko m", p=128))

            # ---- persistent activations ----
            qT_sb = bigpool.tile([128, BS], bf16, tag="qT")   # [2*64 feat, B*S]
            kT_sb = bigpool.tile([128, BS], bf16, tag="kT")
            # v natural layout + ones cols: per 128-row chunk:
            #   [v_h0(0:64) | ones(64) | v_h1(65:129) | ones(129)]
            v_sb = bigpool.tile([128, BS // 128, 130], bf16, tag="v")
            ctxT_sb = bigpool.tile([128, BS], bf16, tag="ctxT")

            nc.vector.memset(v_sb[:, :, 64:65], 1.0)
            nc.vector.memset(v_sb[:, :, 129:130], 1.0)

            # ---- qkv projection chunk (512 tokens) ----
            def emit_qkv_chunk(sc):
                xt = xpool.tile([128, KC, 512], bf16, tag="xt")
                nc.sync.dma_start(xt[:], xT_r[:, :, sc * 512:(sc + 1) * 512])

                ps_qk = psA.tile([128, 1024], f32, tag="psA", name="ps_qk")
                for kk in range(KC):
                    nc.tensor.matmul(ps_qk[:, 0:512], lhsT=wqk_sb[:, kk, 0:128],
                                     rhs=xt[:, kk, :],
                                     start=(kk == 0), stop=(kk == KC - 1))
                for kk in range(KC):
                    nc.tensor.matmul(ps_qk[:, 512:1024], lhsT=wqk_sb[:, kk, 128:256],
                                     rhs=xt[:, kk, :],
                                     start=(kk == 0), stop=(kk == KC - 1))
                qs = slice(sc * 512, (sc + 1) * 512)
                nc.vector.tensor_scalar_add(qT_sb[:, qs], ps_qk[:, 0:512],
                                            bqk_sb[:, 0:1])
                nc.vector.tensor_scalar_add(kT_sb[:, qs], ps_qk[:, 512:1024],
                                            bqk_sb[:, 1:2])

                for s4 in range(4):
                    sidx = sc * 4 + s4
                    ps_v = psB.tile([128, 512], f32, tag="psB", name="ps_v")[:, :128]
                    for kk in range(KC):
                        nc.tensor.matmul(
                            ps_v,
                            lhsT=xt[:, kk, s4 * 128:(s4 + 1) * 128],
                            rhs=wv_sb[:, kk, :],
                            start=(kk == 0), stop=(kk == KC - 1))
                    nc.vector.tensor_tensor(v_sb[:, sidx, 0:64], ps_v[:, 0:64],
                                            bv_sb[:, 0:64], ALU.add)
                    nc.vector.tensor_tensor(v_sb[:, sidx, 65:129], ps_v[:, 64:128],
                                            bv_sb[:, 64:128], ALU.add)

            scope1 = nc.named_scope("qkv"); scope1.__enter__()
            xT_r = xT.rearrange("(ko p) s -> p ko s", p=128)
            for sc in range(SC // 2):
                emit_qkv_chunk(sc)
            qkv_rest = list(range(SC // 2, SC))
            scope1.__exit__(None, None, None)

            scope2 = nc.named_scope("attn"); scope2.__enter__()

            # ---- scores + exp for one (batch, window, head) ----
            # exp tile layout (bf16, cols):
            #   [0 : 2048j)          full k-tile pairs, 512 cols per tile
            #   [2048j : +896)       diag o=0 (512 cols, q>=0), o=1 (384, q>=128)
            #   [2048j+896 : +384)   diag o=2 (256, q>=256), o=3 (128, q>=384)
            def emit_scores_exp(b, j, hl):
                hp = slice(64 * hl, 64 * hl + 64)
                qwin = qT_sb[hp, b * S + j * 512: b * S + (j + 1) * 512]
                et = epool.tile([128, EXPW], bf16, tag=f"exp{hl}", name="exp_t")
                # full tiles, in pairs
                for p in range(2 * j):
                    ps = psA.tile([128, 1024], f32, tag="psA", name="ps_sc")
                    for i in range(2):
                        tt = 2 * p + i
                        kt = b * S + tt * 128
                        nc.tensor.matmul(ps[:, i * 512:(i + 1) * 512],
                                         lhsT=kT_sb[hp, kt:kt + 128], rhs=qwin,
                                         start=True, stop=True)
                    nc.scalar.activation(et[:, p * 1024:(p + 1) * 1024], ps,
                                         ACTF.Exp)
                # diag tiles o=0..3 (keys 128*(4j+o)), trimmed to q >= 128*o
                d0 = 2048 * j
                ps = psA.tile([128, 1024], f32, tag="psA", name="ps_sd")
                for o in range(2):
                    kt = b * S + (4 * j + o) * 128
                    nc.tensor.matmul(ps[:, 512 * o: 896 if o else 512],
                                     lhsT=kT_sb[hp, kt:kt + 128],
                                     rhs=qwin[:, 128 * o:],
                                     start=True, stop=True)
                nc.scalar.activation(et[:, d0:d0 + 896], ps[:, 0:896], ACTF.Exp)
                ps = psA.tile([128, 1024], f32, tag="psA", name="ps_sd")
                for o in range(2, 4):
                    off = 256 * (o - 2)
                    kt = b * S + (4 * j + o) * 128
                    nc.tensor.matmul(ps[:, off: off + 512 - 128 * o],
                                     lhsT=kT_sb[hp, kt:kt + 128],
                                     rhs=qwin[:, 128 * o:],
                                     start=True, stop=True)
                nc.scalar.activation(et[:, d0 + 896:d0 + 1280], ps[:, 0:384],
                                     ACTF.Exp)
                # causal mask on the leading 128 cols of each diag region
                for o in range(4):
                    ro = d0 + (0, 512, 896, 1152)[o]
                    nc.vector.tensor_tensor(et[:, ro:ro + 128], et[:, ro:ro + 128],
                                            mask_sb[:], ALU.mult)
                return et

            # ---- ctx for one (batch, window, head), trimmed ----
            def emit_ctx(b, hl, j, et):
                hp = slice(64 * hl, 64 * hl + 64)
                vcol = slice(65 * hl, 65 * hl + 65)
                pc = psB.tile([128, 512], f32, tag="psB", name="ps_c")
                for tt in range(4 * j):
                    nc.tensor.matmul(
                        pc[:65, :],
                        lhsT=v_sb[:, b * NKT + tt, vcol],
                        rhs=et[:, tt * 512:(tt + 1) * 512],
                        start=(tt == 0), stop=False, skip_group_check=True)
                d0 = 2048 * j
                for o in range(4):
                    ro = d0 + (0, 512, 896, 1152)[o]
                    w = 512 - 128 * o
                    nc.tensor.matmul(
                        pc[:65, 128 * o:],
                        lhsT=v_sb[:, b * NKT + 4 * j + o, vcol],
                        rhs=et[:, ro:ro + w],
                        start=(j == 0 and o == 0), stop=(o == 3),
                        skip_group_check=True)
                cs = slice(b * S + j * 512, b * S + (j + 1) * 512)
                nc.vector.tensor_copy(ctxT_sb[hp, cs], pc[0:64, :])
                den = spool.tile([1, 512], f32, tag="den")
                nc.vector.tensor_copy(den[:], pc[64:65, :])
                recip = spool.tile([1, 512], f32, tag="recip")
                nc.vector.reciprocal_approx_fast(out=recip[:], in_=den[:])
                bcast = spool.tile([128, 512], f32, tag="bcast")
                nc.gpsimd.partition_broadcast(bcast[:], recip[:])
                nc.vector.tensor_tensor(ctxT_sb[hp, cs], ctxT_sb[hp, cs],
                                        bcast[hp, :], ALU.mult)

            # ---- AllToAll for (batch, half) g = 2*b + half ----
            def emit_a2a(g):
                half = g % 2
                b = g // 2
                lo = b * S + half * (S // 2)
                a2a_in_r = a2a_in[g].rearrange("(d p) t -> p d t", p=128)
                nc.sync.dma_start(
                    a2a_in_r[:],
                    ctxT_sb[:, lo: lo + S // 2].rearrange(
                        "p (d t) -> p d t", t=128))
                nc.gpsimd.collective_compute(
                    "AllToAll",
                    mybir.AluOpType.bypass,
                    replica_groups=[list(range(NC))],
                    ins=[a2a_in[g][:]],
                    outs=[a2a_out[g][:]],
                )

            # ---- output projection for (batch, half) g, two stages ----
            # The ctxag DMA is posted one window after the A2A (on the
            # GpSimd queue, which has no later bcasts pending at that
            # point is not guaranteed - but its only other load is the wo
            # DMA and out stores), so by proj-compute time the data AND its
            # completion semaphore have long settled.
            ctxag_tiles = {}

            def prefetch_ctxag(g):
                ctxag_sb = apool.tile([128, NC, 128], bf16, tag="ctxag",
                                      name="ctxag_sb")
                nc.sync.dma_start(
                    ctxag_sb[:], a2a_out[g].rearrange("(d p) t -> p d t", p=128))
                ctxag_tiles[g] = ctxag_sb

            # Proj matmuls are deprioritized (~2 iterations later in the
            # scheduler's logical order) so their collective-gated waits
            # can't head-of-line-block independent attention work in the
            # static per-engine instruction order.
            PRIO_OFF = 0

            def emit_proj(g, prio=PRIO_OFF):
                ctxag_sb = ctxag_tiles.pop(g)
                tc.cur_priority += prio
                ot = opool.tile([128, D], f32, tag="ot")
                for ncol in range(D // 512):
                    ps_o = psP.tile([128, 512], f32, tag="psP", name="ps_o")
                    for k in range(NC):
                        nc.tensor.matmul(
                            ps_o,
                            lhsT=ctxag_sb[:, k, :],
                            rhs=wo_sb[:, k, ncol * 512:(ncol + 1) * 512],
                            start=(k == 0), stop=(k == NC - 1))
                    nc.vector.tensor_tensor(
                        ot[:, ncol * 512:(ncol + 1) * 512], ps_o,
                        bo_sb[:, ncol * 512:(ncol + 1) * 512], ALU.add)
                # tail projs 3 and 1 store via the Scalar queue (idle by
                # then) so the final proj-2 store isn't queued behind them
                eng = nc.scalar if (prio == 0 and g in (3, 1)) else nc.sync
                eng.dma_start(out[g * 128:(g + 1) * 128, :], ot[:])
                tc.cur_priority -= prio

            # ---- main schedule ----
            # Windows per batch in order [2,3,1,0] (batches sequential), so
            # each batch's half1 A2A fires mid-batch and the final A2A is
            # gated on the smallest window (w0). Lag-1 ctx emission; qkv
            # rest-chunks fill batch 0; ctxag prefetch one window after the
            # A2A, proj compute one window after that.
            pending = []

            def emit_ctx_step():
                b, hl, j, et = pending.pop(0)
                emit_ctx(b, hl, j, et)
                if hl == 1 and j in (0, 3):
                    emit_a2a(2 * b + (j == 3))

            WORDER = [2, 3, 1, 0]
            # iteration index -> action after the window's emission
            # a2a(1)@it2, a2a(0)@it4, a2a(3)@it6, a2a(2)@end-flush
            PREF = {3: 1, 5: 0, 7: 3}
            PROJ = {6: 0}
            for it, (b, j) in enumerate((b, j) for b in range(B)
                                        for j in WORDER):
                ets = [emit_scores_exp(b, j, hl) for hl in range(HPC)]
                for hl in range(HPC):
                    pending.append((b, hl, j, ets[hl]))
                while len(pending) > 2:
                    emit_ctx_step()
                if qkv_rest:
                    emit_qkv_chunk(qkv_rest.pop(0))
                if it in PREF:
                    prefetch_ctxag(PREF[it])
                if it in PROJ:
                    emit_proj(PROJ[it])
            while pending:
                emit_ctx_step()
            prefetch_ctxag(2)
            emit_proj(3, prio=0)
            emit_proj(1, prio=0)
            emit_proj(2, prio=0)

            scope2.__exit__(None, None, None)

    nc.compile()
    return nc


def _prep_inputs(x, Wqkv, bqkv, Wo, bo):
    x = np.asarray(x, dtype=np.float32)
    Wqkv = np.asarray(Wqkv, dtype=np.float32)
    bqkv = np.asarray(bqkv, dtype=np.float32)
    Wo = np.asarray(Wo, dtype=np.float32)
    bo = np.asarray(bo, dtype=np.float32)

    xT = np.ascontiguousarray(x.reshape(BS, D).T).astype(BF16)
    wo_b = Wo.astype(BF16)
    bo_t = np.tile(bo.astype(np.float32), (128, 1))

    kp = np.arange(128)[:, None]
    u = np.arange(128)[None, :]
    mask128 = (u >= kp).astype(BF16)

    scale = np.float32(1.0 / np.sqrt(HD))

    W3 = Wqkv.reshape(D, H, 3, HD)
    b3 = bqkv.reshape(H, 3, HD)

    in_maps = []
    for c in range(NC):
        hs = [HPC * c + i for i in range(HPC)]
        wq = np.concatenate([W3[:, h, 0, :] for h in hs], axis=1) * scale
        wk = np.concatenate([W3[:, h, 1, :] for h in hs], axis=1)
        wv_ = np.concatenate([W3[:, h, 2, :] for h in hs], axis=1)
        bq = np.concatenate([b3[h, 0, :] for h in hs]) * scale
        bk = np.concatenate([b3[h, 1, :] for h in hs])
        bv_ = np.concatenate([b3[h, 2, :] for h in hs])
        in_maps.append({
            "xT": xT,
            "wqk": np.ascontiguousarray(
                np.concatenate([wq, wk], axis=1)).astype(BF16),
            "wv": np.ascontiguousarray(wv_).astype(BF16),
            "wo": wo_b,
            "bqk": np.ascontiguousarray(
                np.stack([bq, bk], axis=1)).astype(np.float32),
            "bv": np.tile(bv_.astype(BF16), (128, 1)),
            "bo": bo_t,
            "mask": mask128,
        })
    return in_maps


def run(x, Wqkv, bqkv, Wo, bo, trace=False):
    from concourse.bass_utils import run_bass_kernel_spmd

    if "nc" not in _CACHE:
        _CACHE["nc"] = _build_program()
    nc = _CACHE["nc"]
    in_maps = _prep_inputs(x, Wqkv, bqkv, Wo, bo)
    res = run_bass_kernel_spmd(nc, in_maps, list(range(NC)), trace=trace)
    # core c returns [512, D]: 4 chunks of 128 rows: (b0 half0 rows 128c..),
    # (b0 half1 rows 1024+128c..), (b1 half0), (b1 half1)
    full = np.empty((B, S, D), dtype=np.float32)
    for c in range(NC):
        r = res.results[c]["out"]
        for g in range(4):
            b, half = g // 2, g % 2
            lo = half * 1024 + 128 * c
            full[b, lo:lo + 128, :] = r[g * 128:(g + 1) * 128, :]
    return full, res


def kernel(x, Wqkv, bqkv, Wo, bo):
    out, _ = run(x, Wqkv, bqkv, Wo, bo)
    return out
